# revision 16
# baseline (speedup 1.0000x reference)
"""ALiBi causal attention on 8 TRN2 NeuronCores (Bass/Tile).

Sharding: each core computes HPC=2 heads for BOTH batches (head-parallel,
weights column-sharded).  Scores are computed transposed (S_T[k, q]) so the
ALiBi k-ramp becomes a per-partition fp32 bias applied by the ScalarEngine
exp, and the softmax-invariant q-term is folded into the score matmul as an
extra contraction row.  P@V is computed V-stationary: ctx is accumulated
directly transposed (ctx[c, q]) in a 4-bank PSUM accumulator with wide
moving-operand streams, and a ones-column in V emits softmax denominators
for free.  The scores->exp->P@V cross-engine chain is software-pipelined:
each k-block's P@V matmuls are issued after the NEXT k-block's scores, so
the in-order PE queue always has independent work while the ScalarE exp
runs (4-deep PSUM score buffers).  Normalization broadcasts the bf16
denominator row across partitions with a rank-1 PE matmul, reciprocal on
the broadcast, one multiply per chunk.  One AllToAll per batch; both
batches' compute is issued before either batch's output projection, and a
tiny warm-up collective at t~0 absorbs the one-time CC-channel init.
Compute dtype bf16 (fp32 accumulation in PSUM).
"""

import math

import numpy as np
import ml_dtypes

import bass_rust
import concourse.bass as bass
import concourse.mybir as mybir
import concourse.tile as tile
from concourse.bass_utils import run_bass_kernel_spmd

B, N, D = 2, 2048, 1024
H, HD = 16, 64
NCORES = 8
HPC = H // NCORES      # heads per core = 2
NT = N // 128          # 16 blocks of 128 along seq
NCH = N // 512         # 4 column chunks of 512 along seq
QS = N // 4            # query rows owned per core = 512
QSO = N // NCORES      # query rows owned per core per batch = 256
KT = D // 128          # 8 contraction tiles for d
BF16 = mybir.dt.bfloat16
F32 = mybir.dt.float32
SHIFT = 6.0            # static upper bound of the adjusted logits


def _split_multi_waits(nc):
    """This image's walrus rejects >1 sync-wait per instruction; move extra
    waits onto single-wait NoOps spliced just before the instruction in the
    same engine stream (the engine blocks on the NoOps first)."""
    n_split = 0
    for f in nc.m.functions:
        for bb in f.blocks:
            insts = list(bb.instructions)
            new = []
            for inst in insts:
                si = getattr(inst, "sync_info", None)
                waits = list(si.on_wait) if si is not None and si.on_wait else []
                if len(waits) > 1:
                    for idx, w in enumerate(waits[1:]):
                        nop = mybir.InstNoOp(
                            name=f"{inst.name}-xw{idx}", ins=[], outs=[])
                        nop.engine = inst.engine
                        nop.sync_info = bass_rust.SyncInfo(
                            on_wait=[w], on_update=[])
                        new.append(nop)
                    si.on_wait = waits[:1]
                    n_split += 1
                new.append(inst)
            if len(new) != len(insts):
                bb.instructions = new
    return n_split


def _get_slopes(n):
    def pow2(n):
        start = 2 ** (-(2 ** (-(math.log2(n) - 3))))
        return [start * start**i for i in range(n)]

    if math.log2(n).is_integer():
        return pow2(n)
    c = 2 ** math.floor(math.log2(n))
    return pow2(c) + _get_slopes(2 * c)[0::2][: n - c]


def _blocks_for_kb(kb):
    """1024-aligned block list [(b0, w), ...] covering [kb*128, N)."""
    q0 = kb * 128
    out = []
    b0 = q0
    while b0 < N:
        end = min((b0 // 1024 + 1) * 1024, N)
        out.append((b0, end - b0))
        b0 = end
    return out


def _pieces(lo, hi):
    """Split [lo, hi) at absolute 512 boundaries."""
    out = []
    c = lo
    while c < hi:
        end = min((c // 512 + 1) * 512, hi)
        out.append((c, end - c))
        c = end
    return out


def build_nc():
    nc = bass.Bass()

    xT = nc.declare_dram_parameter("xT", [B, 128, NCH * KT * 512], BF16,
                                   isOutput=False)
    wq = nc.declare_dram_parameter("wq", [128, KT * 128], BF16, isOutput=False)
    wk = nc.declare_dram_parameter("wk", [128, KT * 128], BF16, isOutput=False)
    wv = nc.declare_dram_parameter("wv", [128, KT * 128], BF16, isOutput=False)
    wo = nc.declare_dram_parameter("wo", [128, KT * D], BF16, isOutput=False)
    qrow = nc.declare_dram_parameter("qrow", [HPC, N], BF16, isOutput=False)
    kbias = nc.declare_dram_parameter("kbias", [128, HPC * NT], F32,
                                      isOutput=False)
    maskp = nc.declare_dram_parameter("maskp", [128, 128], BF16,
                                      isOutput=False)
    y = nc.declare_dram_parameter("y", [QS, D], F32, isOutput=True)

    a2a_in = [nc.dram_tensor(f"a2a_in{b}", [NCORES, 128, QSO], BF16)
              for b in range(B)]
    a2a_out = [nc.dram_tensor(f"a2a_out{b}", [NCORES, 128, QSO], BF16)
               for b in range(B)]
    dum_in = nc.dram_tensor("dum_in", [NCORES, 1, 8], BF16)
    dum_out = nc.dram_tensor("dum_out", [NCORES, 1, 8], BF16)
    groups = [list(range(NCORES))]

    from contextlib import ExitStack

    with tile.TileContext(nc) as tc, ExitStack() as est:
        cpool = est.enter_context(tc.tile_pool(name="const", bufs=1))
        xpool = est.enter_context(tc.tile_pool(name="x", bufs=1))
        qkpool = est.enter_context(tc.tile_pool(name="qk", bufs=1))
        vpool = est.enter_context(tc.tile_pool(name="v", bufs=1))
        ppool = est.enter_context(tc.tile_pool(name="p", bufs=6))
        rpool = est.enter_context(tc.tile_pool(name="rc", bufs=2))
        ctpool = est.enter_context(tc.tile_pool(name="ct", bufs=1))
        cfpool = est.enter_context(tc.tile_pool(name="cf", bufs=1))
        opool = est.enter_context(tc.tile_pool(name="ob", bufs=2))
        scps = est.enter_context(tc.tile_pool(name="sc", bufs=2, space="PSUM"))
        ctxps = est.enter_context(tc.tile_pool(name="cx", bufs=1, space="PSUM"))

        # warm-up collective first: kicks the one-time CC-channel init off
        # the critical path (the GpSimd queue carries only collectives)
        ones64 = cpool.tile([1, 64], BF16, tag="ones64", name="ones64")
        nc.vector.memset(ones64[:], 1.0)
        nc.sync.dma_start(out=dum_in[:].rearrange("j p q -> p (j q)"),
                          in_=ones64[:])
        nc.gpsimd.collective_compute(
            "AllToAll", mybir.AluOpType.bypass, replica_groups=groups,
            ins=[dum_in[:].opt()], outs=[dum_out[:].opt()],
        )

        mask = cpool.tile([128, 128], BF16, tag="mask", name="mask")
        nc.sync.dma_start(out=mask[:], in_=maskp[:])
        wq_sb = cpool.tile([128, KT * 128], BF16, tag="wq", name="wq_sb")
        nc.sync.dma_start(out=wq_sb[:], in_=wq[:])
        wk_sb = cpool.tile([128, KT * 128], BF16, tag="wk", name="wk_sb")
        nc.sync.dma_start(out=wk_sb[:], in_=wk[:])
        wv_sb = cpool.tile([128, KT * 128], BF16, tag="wv", name="wv_sb")
        nc.sync.dma_start(out=wv_sb[:], in_=wv[:])
        kb_sb = cpool.tile([128, HPC * NT], F32, tag="kb", name="kb_sb")
        nc.sync.dma_start(out=kb_sb[:], in_=kbias[:])

        # x tiles, chunk-contiguous: cols [ch*KT*512 + kt*512 + j]
        x_t = []
        for b in range(B):
            xt = xpool.tile([128, NCH * KT * 512], BF16, tag=f"xt{b}",
                            name=f"x_t{b}")
            x_t.append(xt)
            for ch in range(NCH):
                cs = slice(ch * KT * 512, (ch + 1) * KT * 512)
                nc.sync.dma_start(out=xt[:, cs], in_=xT[b][:, cs])

        # wo is needed last; keep its 2 MB off the early DMA window
        wo_sb = cpool.tile([128, KT * D], BF16, tag="wo", name="wo_sb")
        nc.sync.dma_start(out=wo_sb[:], in_=wo[:])

        def xcol(b, ch, kt):
            base = ch * KT * 512 + kt * 512
            return x_t[b][:, base:base + 512]

        ct = [ctpool.tile([128, N], BF16, tag=f"ct{b}", name=f"ct{b}")
              for b in range(B)]
        cf = [[cfpool.tile([128, QSO], BF16, tag=f"cf{b}_{i}",
                           name=f"cf{b}_{i}")
               for i in range(NCORES)] for b in range(B)]

        def compute_batch(b):
            # ---- Q/K projections: qe/ke [65, N], rows 0..63 head data,
            # row 64 the extra contraction row (q-term / ones) ----
            qe = [qkpool.tile([65, N], BF16, tag=f"qe{b}{h}", name=f"qe{b}{h}")
                  for h in range(HPC)]
            ke = [qkpool.tile([65, N], BF16, tag=f"ke{b}{h}", name=f"ke{b}{h}")
                  for h in range(HPC)]
            for h in range(HPC):
                nc.sync.dma_start(out=qe[h][64:65, :], in_=qrow[h:h + 1, :])
                nc.vector.memset(ke[h][64:65, :], 1.0)
            for w_sb, dst in ((wq_sb, qe), (wk_sb, ke)):
                for g in range(2):
                    ps = scps.tile([128, 1024], F32, tag="sc", name="ps")
                    for ch in (2 * g, 2 * g + 1):
                        o = (ch % 2) * 512
                        for kt in range(KT):
                            nc.tensor.matmul(
                                ps[:, o:o + 512],
                                lhsT=w_sb[:, kt * 128:(kt + 1) * 128],
                                rhs=xcol(b, ch, kt),
                                start=(kt == 0), stop=(kt == KT - 1),
                            )
                    cs = slice(g * 1024, (g + 1) * 1024)
                    nc.vector.tensor_copy(dst[0][0:64, cs], ps[0:64, :])
                    nc.vector.tensor_copy(dst[1][0:64, cs], ps[64:128, :])

            # ---- V: v_t[nb] [128, 130]; per head 64 value cols + ones col
            # (cols h*65..h*65+64), built with one strided DVE copy ----
            v_t = [vpool.tile([128, HPC * 65], BF16, tag=f"v{b}_{nb}",
                              name=f"v{b}_{nb}")
                   for nb in range(NT)]
            for g in range(4):
                ps = scps.tile([128, 1024], F32, tag="sc", name="vps")
                for j in range(4):
                    nb = 4 * g + j
                    for kt in range(KT):
                        nc.tensor.matmul(
                            ps[:, j * 128:(j + 1) * 128],
                            lhsT=xcol(b, nb // 4, kt)[:, (nb % 4) * 128:
                                                      (nb % 4) * 128 + 128],
                            rhs=wv_sb[:, kt * 128:(kt + 1) * 128],
                            start=(kt == 0), stop=(kt == KT - 1),
                        )
                    vr = v_t[nb][:].rearrange("p (g c) -> p g c", g=HPC)
                    sr = ps[:, j * 128:(j + 1) * 128].rearrange(
                        "p (g c) -> p g c", g=HPC)
                    nc.vector.tensor_copy(vr[:, :, 0:64], sr[:])
                    nc.vector.memset(vr[:, :, 64:65], 1.0)

            # ---- attention (per head): ctx accumulated transposed.
            # Software pipeline: PV(kb) is issued after scores(kb+1) so the
            # in-order PE never head-of-line blocks on the ScalarE exp. ----
            for h in range(HPC):
                ctx = ctxps.tile([128, N], F32, tag="ctx", name="ctx")
                vsl = slice(h * 65, (h + 1) * 65)

                def scores_exp(kb):
                    q0 = kb * 128
                    col = h * NT + kb
                    pts = []
                    for bi, (b0, bw) in enumerate(_blocks_for_kb(kb)):
                        al = (b0 // 1024) * 1024
                        ps = scps.tile([128, 1024], F32, tag="sc", name="sps")
                        for p0, pw in _pieces(b0, b0 + bw):
                            nc.tensor.matmul(
                                ps[:, p0 - al:p0 - al + pw],
                                lhsT=ke[h][:, q0:q0 + 128],
                                rhs=qe[h][:, p0:p0 + pw],
                                start=True, stop=True,
                            )
                        p_t = ppool.tile([128, 1024], BF16, tag="p",
                                         name="p_t")
                        pts.append(p_t)
                        nc.scalar.activation(
                            p_t[:, 0:bw], ps[:, b0 - al:b0 - al + bw],
                            mybir.ActivationFunctionType.Exp,
                            bias=kb_sb[:, col:col + 1], scale=1.0,
                        )
                        if bi == 0:  # causal mask on the diagonal block
                            nc.vector.tensor_tensor(
                                p_t[:, 0:128], p_t[:, 0:128], mask[:],
                                op=mybir.AluOpType.mult,
                            )
                    return pts

                def pv(kb, pts):
                    q0 = kb * 128
                    blocks = _blocks_for_kb(kb)
                    if kb == 0:
                        for bi, (b0, bw) in enumerate(blocks):
                            for p0, pw in _pieces(b0, b0 + bw):
                                nc.tensor.matmul(
                                    ctx[0:65, p0:p0 + pw],
                                    lhsT=v_t[kb][:, vsl],
                                    rhs=pts[bi][:, p0 - b0:p0 - b0 + pw],
                                    start=True, stop=False,
                                    skip_group_check=True,
                                )
                        return
                    for bi, (b0, bw) in enumerate(blocks):
                        lo = b0 + 128 if bi == 0 else b0
                        for p0, pw in _pieces(lo, b0 + bw):
                            nc.tensor.matmul(
                                ctx[0:65, p0:p0 + pw],
                                lhsT=v_t[kb][:, vsl],
                                rhs=pts[bi][:, p0 - b0:p0 - b0 + pw],
                                start=False, stop=False,
                                skip_group_check=True,
                            )
                    nc.tensor.matmul(  # diagonal block: final write
                        ctx[0:65, q0:q0 + 128],
                        lhsT=v_t[kb][:, vsl], rhs=pts[0][:, 0:128],
                        start=False, stop=True,
                        skip_group_check=True,
                    )

                prev = None
                for kb in range(NT):
                    pts = scores_exp(kb)
                    if prev is not None:
                        pv(*prev)
                    prev = (kb, pts)
                pv(*prev)

                # normalize: ct[h rows, q] = ctx[0:64, q] * (1/ctx[64, q]).
                # Denominator row -> SBUF bf16 (ScalarE), broadcast across
                # partitions by a rank-1 bf16 matmul, reciprocal on the
                # broadcast (DVE, PSUM->SBUF), then one multiply per chunk
                # (DVE reads at most one PSUM operand per instruction).
                den = rpool.tile([1, N], BF16, tag="den", name="den")
                nc.scalar.copy(den[:], ctx[64:65, :])
                for c0 in range(0, N, 1024):
                    bc = scps.tile([128, 1024], F32, tag="sc", name="bc")
                    for o in (0, 512):
                        nc.tensor.matmul(
                            bc[0:64, o:o + 512], lhsT=ones64[:],
                            rhs=den[:, c0 + o:c0 + o + 512],
                            start=True, stop=True,
                        )
                    bcr = rpool.tile([64, 1024], F32, tag="bcr", name="bcr")
                    nc.vector.reciprocal(bcr[:], bc[0:64, :])
                    nc.vector.tensor_tensor(
                        ct[b][h * 64:(h + 1) * 64, c0:c0 + 1024],
                        ctx[0:64, c0:c0 + 1024], bcr[:],
                        op=mybir.AluOpType.mult,
                    )
            # ---- stage + AllToAll for this batch ----
            nc.sync.dma_start(
                out=a2a_in[b][:].rearrange("j p q -> p j q"),
                in_=ct[b][:].rearrange("p (j q) -> p j q", j=NCORES),
            )
            nc.gpsimd.collective_compute(
                "AllToAll", mybir.AluOpType.bypass, replica_groups=groups,
                ins=[a2a_in[b][:].opt()], outs=[a2a_out[b][:].opt()],
            )
            for i in range(NCORES):
                nc.sync.dma_start(out=cf[b][i][:], in_=a2a_out[b][i])

        def out_proj(b):
            for q4 in range(QSO // 128):
                ob = opool.tile([128, D], F32, tag="ob", name="ob")
                ps = scps.tile([128, 1024], F32, tag="sc", name="wps")
                for nch in range(D // 512):
                    for kt in range(KT):
                        nc.tensor.matmul(
                            ps[:, nch * 512:(nch + 1) * 512],
                            lhsT=cf[b][kt][:, q4 * 128:(q4 + 1) * 128],
                            rhs=wo_sb[:, kt * D + nch * 512:
                                      kt * D + (nch + 1) * 512],
                            start=(kt == 0), stop=(kt == KT - 1),
                        )
                nc.vector.tensor_copy(ob[:], ps[:])
                r0 = b * QSO + q4 * 128
                nc.sync.dma_start(out=y[r0:r0 + 128, :], in_=ob[:])

        # Both batches' compute first; output projections last so the
        # in-order PE queue never waits on a collective.
        compute_batch(0)
        compute_batch(1)
        out_proj(0)
        out_proj(1)

    _split_multi_waits(nc)
    return nc


_NC_CACHE = None


def _prep_inputs(x, Wq, Wk, Wv, Wo, bo):
    """Host-side sharding/layout prep. Returns in_maps for the 8 cores."""
    bf = ml_dtypes.bfloat16
    x = np.asarray(x, np.float32)
    slopes = np.array(_get_slopes(H), np.float64)

    # x transposed, chunk-contiguous: xTr[b, p, ch*KT*512 + kt*512 + j]
    #   = x[b, ch*512 + j, kt*128 + p]
    xTr = np.ascontiguousarray(
        x.transpose(0, 2, 1)                     # [B, D, N]
        .reshape(B, KT, 128, NCH, 512)
        .transpose(0, 2, 3, 1, 4)                # [B, 128, NCH, KT, 512]
        .reshape(B, 128, NCH * KT * 512)
    ).astype(bf)

    def wtile(w):  # [D, m] -> [128, KT*m]
        m = w.shape[1]
        return np.ascontiguousarray(
            w.reshape(KT, 128, m).transpose(1, 0, 2).reshape(128, KT * m)
        ).astype(bf)

    # causal keep-mask in S_T layout: 1 where k(partition) <= q(free)
    pp = np.arange(128)
    maskv = (pp[:, None] <= pp[None, :]).astype(bf)

    wo_r = wtile(np.asarray(Wo, np.float32))
    in_maps = []
    for c in range(NCORES):
        hs = slice(c * HPC * HD, (c + 1) * HPC * HD)
        sl = slopes[c * HPC:(c + 1) * HPC] / 8.0
        q_idx = np.arange(N, dtype=np.float64)
        qr = (-sl[:, None] * q_idx[None, :] - SHIFT).astype(bf)
        p = np.arange(128, dtype=np.float64)
        kb = np.zeros((128, HPC * NT), np.float32)
        for h in range(HPC):
            for t in range(NT):
                kb[:, h * NT + t] = (sl[h] * (t * 128 + p)).astype(np.float32)
        in_maps.append({
            "xT": xTr,
            "wq": wtile(np.asarray(Wq, np.float32)[:, hs] / 8.0),
            "wk": wtile(np.asarray(Wk, np.float32)[:, hs]),
            "wv": wtile(np.asarray(Wv, np.float32)[:, hs]),
            "wo": wo_r,
            "qrow": qr,
            "kbias": kb,
            "maskp": maskv,
        })
    return in_maps


def _try_device_reset():
    """Best-effort NeuronCore reset via the axon client (clears collective
    state a previously killed run may have left behind)."""
    try:
        import ctypes
        import time as _time

        import jax

        jax.devices()
        lib = ctypes.CDLL("/opt/axon/libaxon_pjrt.so")
        lib.axon_reset.restype = ctypes.c_int64
        lib.axon_reset()
        _time.sleep(5)
    except Exception:
        pass


def kernel(x, Wq, Wk, Wv, Wo, bo):
    global _NC_CACHE
    if _NC_CACHE is None:
        _NC_CACHE = build_nc()
    nc = _NC_CACHE
    in_maps = _prep_inputs(x, Wq, Wk, Wv, Wo, bo)
    try:
        res = run_bass_kernel_spmd(nc, in_maps, list(range(NCORES)))
    except Exception:
        _try_device_reset()
        res = run_bass_kernel_spmd(nc, in_maps, list(range(NCORES)))
    out = np.empty((B, N, D), np.float32)
    for c in range(NCORES):
        for b in range(B):
            out[b, c * QSO:(c + 1) * QSO, :] = \
                res.results[c]["y"][b * QSO:(b + 1) * QSO]
    out += np.asarray(bo, np.float32)[None, None, :]
    return out


# revision 17
# speedup vs baseline: 1.1439x; 1.1439x over previous
"""ALiBi causal attention on 8 TRN2 NeuronCores (Bass/Tile).

Sharding: each core computes HPC=2 heads for BOTH batches (head-parallel,
weights column-sharded).  Scores are computed transposed (S_T[k, q]) so the
ALiBi k-ramp becomes a per-partition fp32 bias applied by the ScalarEngine
exp, and the softmax-invariant q-term is folded into the score matmul as an
extra contraction row.  P@V is computed V-stationary: ctx is accumulated
directly transposed (ctx[c, q]) in a 4-bank PSUM accumulator, and a
ones-column in V emits softmax denominators for free.

The kernel's two walls are the ScalarEngine exp (~1.04 ns/col) during
attention and the PE during projections.  To keep both saturated, batch
1's projections are INTERLEAVED instruction-by-instruction into batch 0's
attention stream (generator-based two-stream emission, separate PSUM
pools), and the scores->exp->P@V chain is software-pipelined (P@V issued
one k-block behind scores).  Batch 1's AllToAll is split per head so the
final collective is covered by batch 0's output projection; a tiny warm-up
collective at t~0 absorbs the one-time CC-channel init.  Compute dtype
bf16 (fp32 accumulation in PSUM).
"""

import math

import numpy as np
import ml_dtypes

import bass_rust
import concourse.bass as bass
import concourse.mybir as mybir
import concourse.tile as tile
from concourse.bass_utils import run_bass_kernel_spmd

B, N, D = 2, 2048, 1024
H, HD = 16, 64
NCORES = 8
HPC = H // NCORES      # heads per core = 2
NT = N // 128          # 16 blocks of 128 along seq
NCH = N // 512         # 4 column chunks of 512 along seq
QS = N // 4            # query rows owned per core = 512
QSO = N // NCORES      # query rows owned per core per batch = 256
KT = D // 128          # 8 contraction tiles for d
BF16 = mybir.dt.bfloat16
F32 = mybir.dt.float32
SHIFT = 6.0            # static upper bound of the adjusted logits


def _split_multi_waits(nc):
    """This image's walrus rejects >1 sync-wait per instruction; move extra
    waits onto single-wait NoOps spliced just before the instruction in the
    same engine stream (the engine blocks on the NoOps first)."""
    n_split = 0
    for f in nc.m.functions:
        for bb in f.blocks:
            insts = list(bb.instructions)
            new = []
            for inst in insts:
                si = getattr(inst, "sync_info", None)
                waits = list(si.on_wait) if si is not None and si.on_wait else []
                if len(waits) > 1:
                    for idx, w in enumerate(waits[1:]):
                        nop = mybir.InstNoOp(
                            name=f"{inst.name}-xw{idx}", ins=[], outs=[])
                        nop.engine = inst.engine
                        nop.sync_info = bass_rust.SyncInfo(
                            on_wait=[w], on_update=[])
                        new.append(nop)
                    si.on_wait = waits[:1]
                    n_split += 1
                new.append(inst)
            if len(new) != len(insts):
                bb.instructions = new
    return n_split


def _get_slopes(n):
    def pow2(n):
        start = 2 ** (-(2 ** (-(math.log2(n) - 3))))
        return [start * start**i for i in range(n)]

    if math.log2(n).is_integer():
        return pow2(n)
    c = 2 ** math.floor(math.log2(n))
    return pow2(c) + _get_slopes(2 * c)[0::2][: n - c]


def _chunks_for_kb(kb):
    """512-aligned chunk list [(c0, cw), ...] covering [kb*128, N)."""
    q0 = kb * 128
    out = []
    c0 = q0
    while c0 < N:
        end = min((c0 // 512 + 1) * 512, N)
        out.append((c0, end - c0))
        c0 = end
    return out


def build_nc():
    nc = bass.Bass()

    xT = nc.declare_dram_parameter("xT", [B, 128, NCH * KT * 512], BF16,
                                   isOutput=False)
    wq = nc.declare_dram_parameter("wq", [128, KT * 128], BF16, isOutput=False)
    wk = nc.declare_dram_parameter("wk", [128, KT * 128], BF16, isOutput=False)
    wv = nc.declare_dram_parameter("wv", [128, KT * 128], BF16, isOutput=False)
    wo = nc.declare_dram_parameter("wo", [128, KT * D], BF16, isOutput=False)
    qrow = nc.declare_dram_parameter("qrow", [HPC, N], BF16, isOutput=False)
    kbias = nc.declare_dram_parameter("kbias", [128, HPC * NT], F32,
                                      isOutput=False)
    maskp = nc.declare_dram_parameter("maskp", [128, 128], BF16,
                                      isOutput=False)
    y = nc.declare_dram_parameter("y", [QS, D], F32, isOutput=True)

    # batch 0: one AllToAll; batch 1: one per head (the last is covered by
    # batch 0's output projection)
    a2a_in0 = nc.dram_tensor("a2a_in0", [NCORES, 128, QSO], BF16)
    a2a_out0 = nc.dram_tensor("a2a_out0", [NCORES, 128, QSO], BF16)
    a2a_in1 = [nc.dram_tensor(f"a2a_in1{h}", [NCORES, 64, QSO], BF16)
               for h in range(HPC)]
    a2a_out1 = [nc.dram_tensor(f"a2a_out1{h}", [NCORES, 64, QSO], BF16)
                for h in range(HPC)]
    dum_in = nc.dram_tensor("dum_in", [NCORES, 1, 8], BF16)
    dum_out = nc.dram_tensor("dum_out", [NCORES, 1, 8], BF16)
    groups = [list(range(NCORES))]

    from contextlib import ExitStack

    with tile.TileContext(nc) as tc, ExitStack() as est:
        cpool = est.enter_context(tc.tile_pool(name="const", bufs=1))
        xpool = est.enter_context(tc.tile_pool(name="x", bufs=1))
        qkpool = est.enter_context(tc.tile_pool(name="qk", bufs=1))
        vpool = est.enter_context(tc.tile_pool(name="v", bufs=1))
        ppool = est.enter_context(tc.tile_pool(name="p", bufs=8))
        rpool = est.enter_context(tc.tile_pool(name="rc", bufs=2))
        ctpool = est.enter_context(tc.tile_pool(name="ct", bufs=1))
        cfpool = est.enter_context(tc.tile_pool(name="cf", bufs=1))
        opool = est.enter_context(tc.tile_pool(name="ob", bufs=2))
        atps = est.enter_context(tc.tile_pool(name="at", bufs=2, space="PSUM"))
        pjps = est.enter_context(tc.tile_pool(name="pj", bufs=2, space="PSUM"))
        ctxps = est.enter_context(tc.tile_pool(name="cx", bufs=1, space="PSUM"))

        # warm-up collective first: kicks the one-time CC-channel init off
        # the critical path (the GpSimd queue carries only collectives)
        ones64 = cpool.tile([1, 64], BF16, tag="ones64", name="ones64")
        nc.vector.memset(ones64[:], 1.0)
        nc.sync.dma_start(out=dum_in[:].rearrange("j p q -> p (j q)"),
                          in_=ones64[:])
        nc.gpsimd.collective_compute(
            "AllToAll", mybir.AluOpType.bypass, replica_groups=groups,
            ins=[dum_in[:].opt()], outs=[dum_out[:].opt()],
        )

        mask = cpool.tile([128, 128], BF16, tag="mask", name="mask")
        nc.sync.dma_start(out=mask[:], in_=maskp[:])
        wq_sb = cpool.tile([128, KT * 128], BF16, tag="wq", name="wq_sb")
        nc.sync.dma_start(out=wq_sb[:], in_=wq[:])
        wk_sb = cpool.tile([128, KT * 128], BF16, tag="wk", name="wk_sb")
        nc.sync.dma_start(out=wk_sb[:], in_=wk[:])
        wv_sb = cpool.tile([128, KT * 128], BF16, tag="wv", name="wv_sb")
        nc.sync.dma_start(out=wv_sb[:], in_=wv[:])
        kb_sb = cpool.tile([128, HPC * NT], F32, tag="kb", name="kb_sb")
        nc.sync.dma_start(out=kb_sb[:], in_=kbias[:])

        # x tiles, chunk-contiguous: cols [ch*KT*512 + kt*512 + j]
        x_t = []
        for b in range(B):
            xt = xpool.tile([128, NCH * KT * 512], BF16, tag=f"xt{b}",
                            name=f"x_t{b}")
            x_t.append(xt)
            for ch in range(NCH):
                cs = slice(ch * KT * 512, (ch + 1) * KT * 512)
                nc.sync.dma_start(out=xt[:, cs], in_=xT[b][:, cs])

        # wo is needed last; keep its 2 MB off the early DMA window
        wo_sb = cpool.tile([128, KT * D], BF16, tag="wo", name="wo_sb")
        nc.sync.dma_start(out=wo_sb[:], in_=wo[:])

        def xcol(b, ch, kt):
            base = ch * KT * 512 + kt * 512
            return x_t[b][:, base:base + 512]

        ct = [ctpool.tile([128, N], BF16, tag=f"ct{b}", name=f"ct{b}")
              for b in range(B)]
        cf = [[cfpool.tile([128, QSO], BF16, tag=f"cf{b}_{i}",
                           name=f"cf{b}_{i}")
               for i in range(NCORES)] for b in range(B)]
        tiles = {}  # b -> (qe, ke, v_t), filled by proj_stream

        def proj_stream(b, act_copies):
            """Projections for batch b.  Yields after ~2 matmuls so it can
            be interleaved with an attention stream."""
            qe = [qkpool.tile([65, N], BF16, tag=f"qe{b}{h}", name=f"qe{b}{h}")
                  for h in range(HPC)]
            ke = [qkpool.tile([65, N], BF16, tag=f"ke{b}{h}", name=f"ke{b}{h}")
                  for h in range(HPC)]
            v_t = [vpool.tile([128, HPC * 65], BF16, tag=f"v{b}_{nb}",
                              name=f"v{b}_{nb}")
                   for nb in range(NT)]
            tiles[b] = (qe, ke, v_t)
            for h in range(HPC):
                nc.sync.dma_start(out=qe[h][64:65, :], in_=qrow[h:h + 1, :])
                nc.vector.memset(ke[h][64:65, :], 1.0)
            cop = nc.scalar.copy if act_copies else nc.vector.tensor_copy
            for w_sb, dst in ((wq_sb, qe), (wk_sb, ke)):
                for ch in range(NCH):
                    ps = pjps.tile([128, 512], F32, tag="pj", name="ps")
                    for kt in range(0, KT, 2):
                        nc.tensor.matmul(
                            ps[:], lhsT=w_sb[:, kt * 128:(kt + 1) * 128],
                            rhs=xcol(b, ch, kt),
                            start=(kt == 0), stop=False,
                        )
                        nc.tensor.matmul(
                            ps[:], lhsT=w_sb[:, (kt + 1) * 128:(kt + 2) * 128],
                            rhs=xcol(b, ch, kt + 1),
                            start=False, stop=(kt == KT - 2),
                        )
                        yield
                    cs = slice(ch * 512, (ch + 1) * 512)
                    cop(dst[0][0:64, cs], ps[0:64, :])
                    cop(dst[1][0:64, cs], ps[64:128, :])
            for nb in range(NT):
                ps = pjps.tile([128, 512], F32, tag="pj", name="vps")
                for kt in range(0, KT, 2):
                    for k2 in (kt, kt + 1):
                        nc.tensor.matmul(
                            ps[:, 0:128],
                            lhsT=xcol(b, nb // 4, k2)[:, (nb % 4) * 128:
                                                      (nb % 4) * 128 + 128],
                            rhs=wv_sb[:, k2 * 128:(k2 + 1) * 128],
                            start=(k2 == 0), stop=(k2 == KT - 1),
                        )
                    yield
                vr = v_t[nb][:].rearrange("p (g c) -> p g c", g=HPC)
                sr = ps[:, 0:128].rearrange("p (g c) -> p g c", g=HPC)
                nc.vector.tensor_copy(vr[:, :, 0:64], sr[:])
                nc.vector.memset(vr[:, :, 64:65], 1.0)

        def attn_stream(b):
            """Attention for batch b.  P@V is issued one k-block behind
            scores (software pipeline); yields after each PE instruction."""
            qe, ke, v_t = tiles[b]
            for h in range(HPC):
                ctx = ctxps.tile([128, N], F32, tag="ctx", name="ctx")
                vsl = slice(h * 65, (h + 1) * 65)

                def scores_exp(kb):
                    q0 = kb * 128
                    col = h * NT + kb
                    pts = []
                    for ci, (c0, cw) in enumerate(_chunks_for_kb(kb)):
                        ps = atps.tile([128, 512], F32, tag="at", name="sps")
                        nc.tensor.matmul(
                            ps[:, 0:cw],
                            lhsT=ke[h][:, q0:q0 + 128],
                            rhs=qe[h][:, c0:c0 + cw],
                            start=True, stop=True,
                        )
                        p_t = ppool.tile([128, 512], BF16, tag="p", name="p_t")
                        pts.append(p_t)
                        nc.scalar.activation(
                            p_t[:, 0:cw], ps[:, 0:cw],
                            mybir.ActivationFunctionType.Exp,
                            bias=kb_sb[:, col:col + 1], scale=1.0,
                        )
                        if ci == 0:  # causal mask on the diagonal block
                            nc.vector.tensor_tensor(
                                p_t[:, 0:128], p_t[:, 0:128], mask[:],
                                op=mybir.AluOpType.mult,
                            )
                        yield pts

                def pv(kb, pts):
                    q0 = kb * 128
                    chunks = _chunks_for_kb(kb)
                    if kb == 0:
                        for ci, (c0, cw) in enumerate(chunks):
                            nc.tensor.matmul(
                                ctx[0:65, c0:c0 + cw],
                                lhsT=v_t[kb][:, vsl], rhs=pts[ci][:, 0:cw],
                                start=True, stop=False,
                                skip_group_check=True,
                            )
                            yield
                        return
                    c00, cw0 = chunks[0]
                    if cw0 > 128:  # first chunk minus the diagonal block
                        nc.tensor.matmul(
                            ctx[0:65, c00 + 128:c00 + cw0],
                            lhsT=v_t[kb][:, vsl], rhs=pts[0][:, 128:cw0],
                            start=False, stop=False,
                            skip_group_check=True,
                        )
                        yield
                    for ci, (c0, cw) in enumerate(chunks[1:], 1):
                        nc.tensor.matmul(
                            ctx[0:65, c0:c0 + cw],
                            lhsT=v_t[kb][:, vsl], rhs=pts[ci][:, 0:cw],
                            start=False, stop=False,
                            skip_group_check=True,
                        )
                        yield
                    nc.tensor.matmul(  # diagonal block: final write
                        ctx[0:65, q0:q0 + 128],
                        lhsT=v_t[kb][:, vsl], rhs=pts[0][:, 0:128],
                        start=False, stop=True,
                        skip_group_check=True,
                    )
                    yield

                prev = None
                for kb in range(NT):
                    pts = None
                    for pts in scores_exp(kb):
                        yield
                    if prev is not None:
                        for _ in pv(*prev):
                            yield
                    prev = (kb, pts)
                for _ in pv(*prev):
                    yield

                # normalize: ct[h rows, q] = ctx[0:64, q] * (1/ctx[64, q])
                den = rpool.tile([1, N], BF16, tag="den", name="den")
                nc.scalar.copy(den[:], ctx[64:65, :])
                for c0 in range(0, N, 512):
                    bc = atps.tile([128, 512], F32, tag="at", name="bc")
                    nc.tensor.matmul(
                        bc[0:64, :], lhsT=ones64[:],
                        rhs=den[:, c0:c0 + 512], start=True, stop=True,
                    )
                    yield
                    bcr = rpool.tile([64, 512], F32, tag="bcr", name="bcr")
                    nc.vector.reciprocal(bcr[:], bc[0:64, :])
                    nc.vector.tensor_tensor(
                        ct[b][h * 64:(h + 1) * 64, c0:c0 + 512],
                        ctx[0:64, c0:c0 + 512], bcr[:],
                        op=mybir.AluOpType.mult,
                    )
                # stage + collective(s)
                if b == 0:
                    if h == HPC - 1:
                        nc.sync.dma_start(
                            out=a2a_in0[:].rearrange("j p q -> p j q"),
                            in_=ct[0][:].rearrange("p (j q) -> p j q",
                                                   j=NCORES),
                        )
                        nc.gpsimd.collective_compute(
                            "AllToAll", mybir.AluOpType.bypass,
                            replica_groups=groups,
                            ins=[a2a_in0[:].opt()], outs=[a2a_out0[:].opt()],
                        )
                        for i in range(NCORES):
                            nc.sync.dma_start(out=cf[0][i][:],
                                              in_=a2a_out0[i])
                else:
                    nc.sync.dma_start(
                        out=a2a_in1[h][:].rearrange("j p q -> p j q"),
                        in_=ct[1][h * 64:(h + 1) * 64, :].rearrange(
                            "p (j q) -> p j q", j=NCORES),
                    )
                    nc.gpsimd.collective_compute(
                        "AllToAll", mybir.AluOpType.bypass,
                        replica_groups=groups,
                        ins=[a2a_in1[h][:].opt()],
                        outs=[a2a_out1[h][:].opt()],
                    )
                    for i in range(NCORES):
                        nc.sync.dma_start(
                            out=cf[1][i][h * 64:(h + 1) * 64, :],
                            in_=a2a_out1[h][i])

        def out_proj(b):
            for q4 in range(QSO // 128):
                ob = opool.tile([128, D], F32, tag="ob", name="ob")
                for nch in range(D // 512):
                    ps = pjps.tile([128, 512], F32, tag="pj", name="wps")
                    for kt in range(KT):
                        nc.tensor.matmul(
                            ps[:],
                            lhsT=cf[b][kt][:, q4 * 128:(q4 + 1) * 128],
                            rhs=wo_sb[:, kt * D + nch * 512:
                                      kt * D + (nch + 1) * 512],
                            start=(kt == 0), stop=(kt == KT - 1),
                        )
                    nc.vector.tensor_copy(
                        ob[:, nch * 512:(nch + 1) * 512], ps[:])
                r0 = b * QSO + q4 * 128
                nc.sync.dma_start(out=y[r0:r0 + 128, :], in_=ob[:])

        def drain(g):
            for _ in g:
                pass

        def interleave(a, p):
            """Alternate emission between two streams until both finish."""
            live = [a, p]
            while live:
                for g in list(live):
                    try:
                        next(g)
                    except StopIteration:
                        live.remove(g)

        drain(proj_stream(0, act_copies=True))
        interleave(attn_stream(0), proj_stream(1, act_copies=False))
        drain(attn_stream(1))
        out_proj(0)   # covers batch 1's last collective
        out_proj(1)

    _split_multi_waits(nc)
    return nc


_NC_CACHE = None


def _prep_inputs(x, Wq, Wk, Wv, Wo, bo):
    """Host-side sharding/layout prep. Returns in_maps for the 8 cores."""
    bf = ml_dtypes.bfloat16
    x = np.asarray(x, np.float32)
    slopes = np.array(_get_slopes(H), np.float64)

    # x transposed, chunk-contiguous: xTr[b, p, ch*KT*512 + kt*512 + j]
    #   = x[b, ch*512 + j, kt*128 + p]
    xTr = np.ascontiguousarray(
        x.transpose(0, 2, 1)                     # [B, D, N]
        .reshape(B, KT, 128, NCH, 512)
        .transpose(0, 2, 3, 1, 4)                # [B, 128, NCH, KT, 512]
        .reshape(B, 128, NCH * KT * 512)
    ).astype(bf)

    def wtile(w):  # [D, m] -> [128, KT*m]
        m = w.shape[1]
        return np.ascontiguousarray(
            w.reshape(KT, 128, m).transpose(1, 0, 2).reshape(128, KT * m)
        ).astype(bf)

    # causal keep-mask in S_T layout: 1 where k(partition) <= q(free)
    pp = np.arange(128)
    maskv = (pp[:, None] <= pp[None, :]).astype(bf)

    wo_r = wtile(np.asarray(Wo, np.float32))
    in_maps = []
    for c in range(NCORES):
        hs = slice(c * HPC * HD, (c + 1) * HPC * HD)
        sl = slopes[c * HPC:(c + 1) * HPC] / 8.0
        q_idx = np.arange(N, dtype=np.float64)
        qr = (-sl[:, None] * q_idx[None, :] - SHIFT).astype(bf)
        p = np.arange(128, dtype=np.float64)
        kb = np.zeros((128, HPC * NT), np.float32)
        for h in range(HPC):
            for t in range(NT):
                kb[:, h * NT + t] = (sl[h] * (t * 128 + p)).astype(np.float32)
        in_maps.append({
            "xT": xTr,
            "wq": wtile(np.asarray(Wq, np.float32)[:, hs] / 8.0),
            "wk": wtile(np.asarray(Wk, np.float32)[:, hs]),
            "wv": wtile(np.asarray(Wv, np.float32)[:, hs]),
            "wo": wo_r,
            "qrow": qr,
            "kbias": kb,
            "maskp": maskv,
        })
    return in_maps


def _try_device_reset():
    """Best-effort NeuronCore reset via the axon client (clears collective
    state a previously killed run may have left behind)."""
    try:
        import ctypes
        import time as _time

        import jax

        jax.devices()
        lib = ctypes.CDLL("/opt/axon/libaxon_pjrt.so")
        lib.axon_reset.restype = ctypes.c_int64
        lib.axon_reset()
        _time.sleep(5)
    except Exception:
        pass


def kernel(x, Wq, Wk, Wv, Wo, bo):
    global _NC_CACHE
    if _NC_CACHE is None:
        _NC_CACHE = build_nc()
    nc = _NC_CACHE
    in_maps = _prep_inputs(x, Wq, Wk, Wv, Wo, bo)
    try:
        res = run_bass_kernel_spmd(nc, in_maps, list(range(NCORES)))
    except Exception:
        _try_device_reset()
        res = run_bass_kernel_spmd(nc, in_maps, list(range(NCORES)))
    out = np.empty((B, N, D), np.float32)
    for c in range(NCORES):
        for b in range(B):
            out[b, c * QSO:(c + 1) * QSO, :] = \
                res.results[c]["y"][b * QSO:(b + 1) * QSO]
    out += np.asarray(bo, np.float32)[None, None, :]
    return out


# revision 30
# speedup vs baseline: 1.2561x; 1.0981x over previous
"""ALiBi causal attention on 8 TRN2 NeuronCores (Bass/Tile).

Sharding: each core computes HPC=2 heads for BOTH batches (head-parallel,
weights column-sharded).  Scores are computed transposed (S_T[k, q]) so the
ALiBi k-ramp becomes a per-partition fp32 bias applied by the ScalarEngine
exp, and the softmax-invariant q-term is folded into the score matmul as an
extra contraction row.  P@V is computed V-stationary: ctx is accumulated
directly transposed (ctx[c, q]) in a 4-bank PSUM accumulator, and a
ones-column in V emits softmax denominators for free.

The kernel's two walls are the ScalarEngine exp (~1.04 ns/col) during
attention and the PE during projections.  To keep both saturated, batch
1's projections are INTERLEAVED instruction-by-instruction into batch 0's
attention stream (generator-based two-stream emission, separate PSUM
pools), and the scores->exp->P@V chain is software-pipelined (P@V issued
one k-block behind scores).  Batch 1's AllToAll is split per head so the
final collective is covered by batch 0's output projection; a tiny warm-up
collective at t~0 absorbs the one-time CC-channel init.  Compute dtype
bf16 (fp32 accumulation in PSUM).
"""

import math

import numpy as np
import ml_dtypes

import bass_rust
import concourse.bass as bass
import concourse.mybir as mybir
import concourse.tile as tile
from concourse.bass_utils import run_bass_kernel_spmd

B, N, D = 2, 2048, 1024
H, HD = 16, 64
NCORES = 8
HPC = H // NCORES      # heads per core = 2
NT = N // 128          # 16 blocks of 128 along seq
NCH = N // 512         # 4 column chunks of 512 along seq
QS = N // 4            # query rows owned per core = 512
QSO = N // NCORES      # query rows owned per core per batch = 256
KT = D // 128          # 8 contraction tiles for d
BF16 = mybir.dt.bfloat16
F32 = mybir.dt.float32
SHIFT = 6.0            # static upper bound of the adjusted logits


def _split_multi_waits(nc):
    """This image's walrus rejects >1 sync-wait per instruction; move extra
    waits onto single-wait NoOps spliced just before the instruction in the
    same engine stream (the engine blocks on the NoOps first)."""
    n_split = 0
    for f in nc.m.functions:
        for bb in f.blocks:
            insts = list(bb.instructions)
            new = []
            for inst in insts:
                si = getattr(inst, "sync_info", None)
                waits = list(si.on_wait) if si is not None and si.on_wait else []
                if len(waits) > 1:
                    for idx, w in enumerate(waits[1:]):
                        nop = mybir.InstNoOp(
                            name=f"{inst.name}-xw{idx}", ins=[], outs=[])
                        nop.engine = inst.engine
                        nop.sync_info = bass_rust.SyncInfo(
                            on_wait=[w], on_update=[])
                        new.append(nop)
                    si.on_wait = waits[:1]
                    n_split += 1
                new.append(inst)
            if len(new) != len(insts):
                bb.instructions = new
    return n_split


def _get_slopes(n):
    def pow2(n):
        start = 2 ** (-(2 ** (-(math.log2(n) - 3))))
        return [start * start**i for i in range(n)]

    if math.log2(n).is_integer():
        return pow2(n)
    c = 2 ** math.floor(math.log2(n))
    return pow2(c) + _get_slopes(2 * c)[0::2][: n - c]


def _chunks_for_kb(kb):
    """512-aligned chunk list [(c0, cw), ...] covering [kb*128, N)."""
    q0 = kb * 128
    out = []
    c0 = q0
    while c0 < N:
        end = min((c0 // 512 + 1) * 512, N)
        out.append((c0, end - c0))
        c0 = end
    return out


def build_nc():
    nc = bass.Bass()

    xT = nc.declare_dram_parameter("xT", [B, 128, NCH * KT * 512], BF16,
                                   isOutput=False)
    wq = nc.declare_dram_parameter("wq", [128, KT * 128], BF16, isOutput=False)
    wk = nc.declare_dram_parameter("wk", [128, KT * 128], BF16, isOutput=False)
    wv = nc.declare_dram_parameter("wv", [128, KT * 128], BF16, isOutput=False)
    wo = nc.declare_dram_parameter("wo", [128, KT * D], BF16, isOutput=False)
    qrow = nc.declare_dram_parameter("qrow", [HPC, N], BF16, isOutput=False)
    kbias = nc.declare_dram_parameter("kbias", [128, HPC * NT], F32,
                                      isOutput=False)
    maskp = nc.declare_dram_parameter("maskp", [128, 128], BF16,
                                      isOutput=False)
    y = nc.declare_dram_parameter("y", [QS, D], F32, isOutput=True)

    # batch 0: one AllToAll; batch 1: one per head (the last is covered by
    # batch 0's output projection)
    a2a_in0 = nc.dram_tensor("a2a_in0", [NCORES, 128, QSO], BF16)
    a2a_out0 = nc.dram_tensor("a2a_out0", [NCORES, 128, QSO], BF16)
    a2a_in1 = [nc.dram_tensor(f"a2a_in1{h}", [NCORES, 64, QSO], BF16)
               for h in range(HPC)]
    a2a_out1 = [nc.dram_tensor(f"a2a_out1{h}", [NCORES, 64, QSO], BF16)
                for h in range(HPC)]
    dum_in = nc.dram_tensor("dum_in", [NCORES, 1, 8], BF16)
    dum_out = nc.dram_tensor("dum_out", [NCORES, 1, 8], BF16)
    # DRAM scratch for the denominator-row reshape (SBUF APs cannot merge
    # partition and free dims; DRAM APs are unconstrained)
    sden = [[nc.dram_tensor(f"sden{b}{h}", [1, N], BF16) for h in range(HPC)]
            for b in range(B)]
    srec = [[nc.dram_tensor(f"srec{b}{h}", [1, N], BF16) for h in range(HPC)]
            for b in range(B)]
    groups = [list(range(NCORES))]

    from contextlib import ExitStack

    with tile.TileContext(nc) as tc, ExitStack() as est:
        cpool = est.enter_context(tc.tile_pool(name="const", bufs=1))
        xpool = est.enter_context(tc.tile_pool(name="x", bufs=1))
        qkpool = est.enter_context(tc.tile_pool(name="qk", bufs=1))
        vpool = est.enter_context(tc.tile_pool(name="v", bufs=1))
        ppool = est.enter_context(tc.tile_pool(name="p", bufs=8))
        rpool = est.enter_context(tc.tile_pool(name="rc", bufs=2))
        ctpool = est.enter_context(tc.tile_pool(name="ct", bufs=1))
        cfpool = est.enter_context(tc.tile_pool(name="cf", bufs=1))
        opool = est.enter_context(tc.tile_pool(name="ob", bufs=2))
        atps = est.enter_context(tc.tile_pool(name="at", bufs=2, space="PSUM"))
        pjps = est.enter_context(tc.tile_pool(name="pj", bufs=2, space="PSUM"))
        ctxps = est.enter_context(tc.tile_pool(name="cx", bufs=1, space="PSUM"))

        # warm-up collective first: kicks the one-time CC-channel init off
        # the critical path (the GpSimd queue carries only collectives)
        ones64 = cpool.tile([1, 64], BF16, tag="ones64", name="ones64")
        nc.vector.memset(ones64[:], 1.0)
        nc.sync.dma_start(out=dum_in[:].rearrange("j p q -> p (j q)"),
                          in_=ones64[:])
        nc.gpsimd.collective_compute(
            "AllToAll", mybir.AluOpType.bypass, replica_groups=groups,
            ins=[dum_in[:].opt()], outs=[dum_out[:].opt()],
        )

        mask = cpool.tile([128, 128], BF16, tag="mask", name="mask")
        nc.sync.dma_start(out=mask[:], in_=maskp[:])
        wq_sb = cpool.tile([128, KT * 128], BF16, tag="wq", name="wq_sb")
        nc.sync.dma_start(out=wq_sb[:], in_=wq[:])
        wk_sb = cpool.tile([128, KT * 128], BF16, tag="wk", name="wk_sb")
        nc.sync.dma_start(out=wk_sb[:], in_=wk[:])
        wv_sb = cpool.tile([128, KT * 128], BF16, tag="wv", name="wv_sb")
        nc.sync.dma_start(out=wv_sb[:], in_=wv[:])
        kb_sb = cpool.tile([128, HPC * NT], F32, tag="kb", name="kb_sb")
        nc.sync.dma_start(out=kb_sb[:], in_=kbias[:])

        # x tiles, chunk-contiguous: cols [ch*KT*512 + kt*512 + j]
        x_t = []
        for b in range(B):
            xt = xpool.tile([128, NCH * KT * 512], BF16, tag=f"xt{b}",
                            name=f"x_t{b}")
            x_t.append(xt)
            for ch in range(NCH):
                cs = slice(ch * KT * 512, (ch + 1) * KT * 512)
                nc.sync.dma_start(out=xt[:, cs], in_=xT[b][:, cs])

        # wo is needed last; keep its 2 MB off the early DMA window
        wo_sb = cpool.tile([128, KT * D], BF16, tag="wo", name="wo_sb")
        nc.sync.dma_start(out=wo_sb[:], in_=wo[:])

        def xcol(b, ch, kt):
            base = ch * KT * 512 + kt * 512
            return x_t[b][:, base:base + 512]

        ct = [ctpool.tile([128, N], BF16, tag=f"ct{b}", name=f"ct{b}")
              for b in range(B)]
        cf = [cfpool.tile([128, NCORES * QSO], BF16, tag=f"cf{b}",
                          name=f"cf{b}")
              for b in range(B)]
        tiles = {}  # b -> (qe, ke, v_t), filled by proj_stream

        def proj_stream(b, act_copies):
            """Projections for batch b.  Yields after ~2 matmuls so it can
            be interleaved with an attention stream."""
            qe = [qkpool.tile([65, N], BF16, tag=f"qe{b}{h}", name=f"qe{b}{h}")
                  for h in range(HPC)]
            ke = [qkpool.tile([65, N], BF16, tag=f"ke{b}{h}", name=f"ke{b}{h}")
                  for h in range(HPC)]
            v_t = [vpool.tile([128, HPC * 65], BF16, tag=f"v{b}_{nb}",
                              name=f"v{b}_{nb}")
                   for nb in range(NT)]
            tiles[b] = (qe, ke, v_t)
            for h in range(HPC):
                nc.sync.dma_start(out=qe[h][64:65, :], in_=qrow[h:h + 1, :])
                nc.vector.memset(ke[h][64:65, :], 1.0)
            cop = nc.scalar.copy if act_copies else nc.vector.tensor_copy
            for w_sb, dst in ((wq_sb, qe), (wk_sb, ke)):
                for ch in range(NCH):
                    ps = pjps.tile([128, 512], F32, tag="pj", name="ps")
                    for kt in range(0, KT, 2):
                        nc.tensor.matmul(
                            ps[:], lhsT=w_sb[:, kt * 128:(kt + 1) * 128],
                            rhs=xcol(b, ch, kt),
                            start=(kt == 0), stop=False,
                        )
                        nc.tensor.matmul(
                            ps[:], lhsT=w_sb[:, (kt + 1) * 128:(kt + 2) * 128],
                            rhs=xcol(b, ch, kt + 1),
                            start=False, stop=(kt == KT - 2),
                        )
                        yield
                    cs = slice(ch * 512, (ch + 1) * 512)
                    cop(dst[0][0:64, cs], ps[0:64, :])
                    cop(dst[1][0:64, cs], ps[64:128, :])
            for nb in range(NT):
                ps = pjps.tile([128, 512], F32, tag="pj", name="vps")
                for kt in range(0, KT, 2):
                    for k2 in (kt, kt + 1):
                        nc.tensor.matmul(
                            ps[:, 0:128],
                            lhsT=xcol(b, nb // 4, k2)[:, (nb % 4) * 128:
                                                      (nb % 4) * 128 + 128],
                            rhs=wv_sb[:, k2 * 128:(k2 + 1) * 128],
                            start=(k2 == 0), stop=(k2 == KT - 1),
                        )
                    yield
                vr = v_t[nb][:].rearrange("p (g c) -> p g c", g=HPC)
                sr = ps[:, 0:128].rearrange("p (g c) -> p g c", g=HPC)
                nc.vector.tensor_copy(vr[:, :, 0:64], sr[:])
                nc.vector.memset(vr[:, :, 64:65], 1.0)

        def attn_stream(b):
            """Attention for batch b.  P@V is issued one k-block behind
            scores (software pipeline); yields after each PE instruction."""
            qe, ke, v_t = tiles[b]
            for h in range(HPC):
                ctx = ctxps.tile([128, N], F32, tag="ctx", name="ctx")
                vsl = slice(h * 65, (h + 1) * 65)

                def scores_exp(kb):
                    q0 = kb * 128
                    col = h * NT + kb
                    pts = []
                    for ci, (c0, cw) in enumerate(_chunks_for_kb(kb)):
                        ps = atps.tile([128, 512], F32, tag="at", name="sps")
                        nc.tensor.matmul(
                            ps[:, 0:cw],
                            lhsT=ke[h][:, q0:q0 + 128],
                            rhs=qe[h][:, c0:c0 + cw],
                            start=True, stop=True,
                        )
                        p_t = ppool.tile([128, 512], BF16, tag="p", name="p_t")
                        pts.append(p_t)
                        nc.scalar.activation(
                            p_t[:, 0:cw], ps[:, 0:cw],
                            mybir.ActivationFunctionType.Exp,
                            bias=kb_sb[:, col:col + 1], scale=1.0,
                        )
                        if ci == 0:  # causal mask on the diagonal block
                            nc.vector.tensor_tensor(
                                p_t[:, 0:128], p_t[:, 0:128], mask[:],
                                op=mybir.AluOpType.mult,
                            )
                        yield pts

                def pv(kb, pts):
                    q0 = kb * 128
                    chunks = _chunks_for_kb(kb)
                    if kb == 0:
                        for ci, (c0, cw) in enumerate(chunks):
                            nc.tensor.matmul(
                                ctx[0:65, c0:c0 + cw],
                                lhsT=v_t[kb][:, vsl], rhs=pts[ci][:, 0:cw],
                                start=True, stop=False,
                                skip_group_check=True,
                            )
                            yield
                        return
                    c00, cw0 = chunks[0]
                    if cw0 > 128:  # first chunk minus the diagonal block
                        nc.tensor.matmul(
                            ctx[0:65, c00 + 128:c00 + cw0],
                            lhsT=v_t[kb][:, vsl], rhs=pts[0][:, 128:cw0],
                            start=False, stop=False,
                            skip_group_check=True,
                        )
                        yield
                    for ci, (c0, cw) in enumerate(chunks[1:], 1):
                        nc.tensor.matmul(
                            ctx[0:65, c0:c0 + cw],
                            lhsT=v_t[kb][:, vsl], rhs=pts[ci][:, 0:cw],
                            start=False, stop=False,
                            skip_group_check=True,
                        )
                        yield
                    nc.tensor.matmul(  # diagonal block: final write
                        ctx[0:65, q0:q0 + 128],
                        lhsT=v_t[kb][:, vsl], rhs=pts[0][:, 0:128],
                        start=False, stop=True,
                        skip_group_check=True,
                    )
                    yield

                prev = None
                for kb in range(NT):
                    pts = None
                    for pts in scores_exp(kb):
                        yield
                    if prev is not None:
                        for _ in pv(*prev):
                            yield
                    prev = (kb, pts)
                for _ in pv(*prev):
                    yield

                # normalize: ct[h rows, q] = ctx[0:64, q] * (1/ctx[64, q]).
                # The DVE's iterative-divide reciprocal costs ~8 cyc per
                # FREE-dim element, so fold the 2048 denominators to
                # [128, 16] with a tiny DMA first (recip there is ~130 ns,
                # not 13 us), DMA back to a row, then broadcast via a
                # rank-1 matmul (projection-pool PSUM so the attention
                # score pipeline is not coupled to extraction), stage to
                # SBUF and multiply.
                den = rpool.tile([1, N], BF16, tag="den", name="den")
                nc.scalar.copy(den[:], ctx[64:65, :])
                nc.sync.dma_start(out=sden[b][h][:], in_=den[:])
                dr = rpool.tile([128, 16], BF16, tag="dr", name="dr")
                nc.sync.dma_start(
                    out=dr[:],
                    in_=sden[b][h][:].rearrange("o (p j) -> (o p) j", p=128),
                )
                rr32 = rpool.tile([128, 16], F32, tag="rr32", name="rr32")
                nc.vector.reciprocal(rr32[:], dr[:])
                rr16 = rpool.tile([128, 16], BF16, tag="rr16", name="rr16")
                nc.vector.tensor_copy(rr16[:], rr32[:])
                nc.sync.dma_start(
                    out=srec[b][h][:].rearrange("o (p j) -> (o p) j", p=128),
                    in_=rr16[:],
                )
                rrow = rpool.tile([1, N], BF16, tag="rrow", name="rrow")
                nc.sync.dma_start(out=rrow[:], in_=srec[b][h][:])
                for c0 in range(0, N, 512):
                    bc = pjps.tile([128, 512], F32, tag="pj", name="bc")
                    nc.tensor.matmul(
                        bc[0:64, :], lhsT=ones64[:],
                        rhs=rrow[:, c0:c0 + 512], start=True, stop=True,
                    )
                    yield
                    bcs = rpool.tile([64, 512], BF16, tag="bcs", name="bcs")
                    nc.vector.tensor_copy(bcs[:], bc[0:64, :])
                    nc.vector.tensor_tensor(
                        ct[b][h * 64:(h + 1) * 64, c0:c0 + 512],
                        ctx[0:64, c0:c0 + 512], bcs[:],
                        op=mybir.AluOpType.mult,
                    )
                # stage + collective(s)
                if b == 0:
                    if h == HPC - 1:
                        nc.sync.dma_start(
                            out=a2a_in0[:].rearrange("j p q -> p j q"),
                            in_=ct[0][:].rearrange("p (j q) -> p j q",
                                                   j=NCORES),
                        )
                        nc.gpsimd.collective_compute(
                            "AllToAll", mybir.AluOpType.bypass,
                            replica_groups=groups,
                            ins=[a2a_in0[:].opt()], outs=[a2a_out0[:].opt()],
                        )
                        nc.sync.dma_start(
                            out=cf[0][:].rearrange("p (j q) -> p j q",
                                                   j=NCORES),
                            in_=a2a_out0[:].rearrange("j p q -> p j q"),
                        )
                else:
                    nc.sync.dma_start(
                        out=a2a_in1[h][:].rearrange("j p q -> p j q"),
                        in_=ct[1][h * 64:(h + 1) * 64, :].rearrange(
                            "p (j q) -> p j q", j=NCORES),
                    )
                    nc.gpsimd.collective_compute(
                        "AllToAll", mybir.AluOpType.bypass,
                        replica_groups=groups,
                        ins=[a2a_in1[h][:].opt()],
                        outs=[a2a_out1[h][:].opt()],
                    )
                    nc.sync.dma_start(
                        out=cf[1][h * 64:(h + 1) * 64, :].rearrange(
                            "p (j q) -> p j q", j=NCORES),
                        in_=a2a_out1[h][:].rearrange("j p q -> p j q"),
                    )

        def out_proj(b):
            for q4 in range(QSO // 128):
                ob = opool.tile([128, D], F32, tag="ob", name="ob")
                for nch in range(D // 512):
                    ps = pjps.tile([128, 512], F32, tag="pj", name="wps")
                    for kt in range(KT):
                        nc.tensor.matmul(
                            ps[:],
                            lhsT=cf[b][:, kt * QSO + q4 * 128:
                                       kt * QSO + (q4 + 1) * 128],
                            rhs=wo_sb[:, kt * D + nch * 512:
                                      kt * D + (nch + 1) * 512],
                            start=(kt == 0), stop=(kt == KT - 1),
                        )
                    nc.vector.tensor_copy(
                        ob[:, nch * 512:(nch + 1) * 512], ps[:])
                r0 = b * QSO + q4 * 128
                nc.sync.dma_start(out=y[r0:r0 + 128, :], in_=ob[:])

        def drain(g):
            for _ in g:
                pass

        def interleave(a, p, ratio=3):
            """Pull `ratio` steps from stream a per step of stream p (the
            attention stream needs ~3 PE instructions issued per exp to keep
            the ScalarE saturated); drain whichever stream survives."""
            a_done = p_done = False
            while not (a_done and p_done):
                for _ in range(ratio):
                    if not a_done:
                        try:
                            next(a)
                        except StopIteration:
                            a_done = True
                if not p_done:
                    try:
                        next(p)
                    except StopIteration:
                        p_done = True

        drain(proj_stream(0, act_copies=True))
        interleave(attn_stream(0), proj_stream(1, act_copies=False))
        drain(attn_stream(1))
        out_proj(0)   # covers batch 1's last collective
        out_proj(1)

    _split_multi_waits(nc)
    return nc


_NC_CACHE = None


def _prep_inputs(x, Wq, Wk, Wv, Wo, bo):
    """Host-side sharding/layout prep. Returns in_maps for the 8 cores."""
    bf = ml_dtypes.bfloat16
    x = np.asarray(x, np.float32)
    slopes = np.array(_get_slopes(H), np.float64)

    # x transposed, chunk-contiguous: xTr[b, p, ch*KT*512 + kt*512 + j]
    #   = x[b, ch*512 + j, kt*128 + p]
    xTr = np.ascontiguousarray(
        x.transpose(0, 2, 1)                     # [B, D, N]
        .reshape(B, KT, 128, NCH, 512)
        .transpose(0, 2, 3, 1, 4)                # [B, 128, NCH, KT, 512]
        .reshape(B, 128, NCH * KT * 512)
    ).astype(bf)

    def wtile(w):  # [D, m] -> [128, KT*m]
        m = w.shape[1]
        return np.ascontiguousarray(
            w.reshape(KT, 128, m).transpose(1, 0, 2).reshape(128, KT * m)
        ).astype(bf)

    # causal keep-mask in S_T layout: 1 where k(partition) <= q(free)
    pp = np.arange(128)
    maskv = (pp[:, None] <= pp[None, :]).astype(bf)

    wo_r = wtile(np.asarray(Wo, np.float32))
    in_maps = []
    for c in range(NCORES):
        hs = slice(c * HPC * HD, (c + 1) * HPC * HD)
        sl = slopes[c * HPC:(c + 1) * HPC] / 8.0
        q_idx = np.arange(N, dtype=np.float64)
        qr = (-sl[:, None] * q_idx[None, :] - SHIFT).astype(bf)
        p = np.arange(128, dtype=np.float64)
        kb = np.zeros((128, HPC * NT), np.float32)
        for h in range(HPC):
            for t in range(NT):
                kb[:, h * NT + t] = (sl[h] * (t * 128 + p)).astype(np.float32)
        in_maps.append({
            "xT": xTr,
            "wq": wtile(np.asarray(Wq, np.float32)[:, hs] / 8.0),
            "wk": wtile(np.asarray(Wk, np.float32)[:, hs]),
            "wv": wtile(np.asarray(Wv, np.float32)[:, hs]),
            "wo": wo_r,
            "qrow": qr,
            "kbias": kb,
            "maskp": maskv,
        })
    return in_maps


def _try_device_reset():
    """Best-effort NeuronCore reset via the axon client (clears collective
    state a previously killed run may have left behind)."""
    try:
        import ctypes
        import time as _time

        import jax

        jax.devices()
        lib = ctypes.CDLL("/opt/axon/libaxon_pjrt.so")
        lib.axon_reset.restype = ctypes.c_int64
        lib.axon_reset()
        _time.sleep(5)
    except Exception:
        pass


def kernel(x, Wq, Wk, Wv, Wo, bo):
    global _NC_CACHE
    if _NC_CACHE is None:
        _NC_CACHE = build_nc()
    nc = _NC_CACHE
    in_maps = _prep_inputs(x, Wq, Wk, Wv, Wo, bo)
    try:
        res = run_bass_kernel_spmd(nc, in_maps, list(range(NCORES)))
    except Exception:
        _try_device_reset()
        res = run_bass_kernel_spmd(nc, in_maps, list(range(NCORES)))
    out = np.empty((B, N, D), np.float32)
    for c in range(NCORES):
        for b in range(B):
            out[b, c * QSO:(c + 1) * QSO, :] = \
                res.results[c]["y"][b * QSO:(b + 1) * QSO]
    out += np.asarray(bo, np.float32)[None, None, :]
    return out


# revision 34
# speedup vs baseline: 1.2942x; 1.0304x over previous
"""ALiBi causal attention on 8 TRN2 NeuronCores (Bass/Tile).

Sharding: each core computes HPC=2 heads for BOTH batches (head-parallel,
weights column-sharded).  Scores are computed transposed (S_T[k, q]) so the
ALiBi k-ramp becomes a per-partition fp32 bias applied by the ScalarEngine
exp, and the softmax-invariant q-term is folded into the score matmul as an
extra contraction row.  P@V is computed V-stationary: ctx is accumulated
directly transposed (ctx[c, q]) in a 4-bank PSUM accumulator, and a
ones-column in V emits softmax denominators for free.

The kernel's two walls are the ScalarEngine exp (~1.04 ns/col) during
attention and the PE during projections.  To keep both saturated, batch
1's projections are INTERLEAVED instruction-by-instruction into batch 0's
attention stream (generator-based two-stream emission, separate PSUM
pools), and the scores->exp->P@V chain is software-pipelined (P@V issued
one k-block behind scores).  Batch 1's AllToAll is split per head so the
final collective is covered by batch 0's output projection; a tiny warm-up
collective at t~0 absorbs the one-time CC-channel init.  Compute dtype
bf16 (fp32 accumulation in PSUM).
"""

import math

import numpy as np
import ml_dtypes

import bass_rust
import concourse.bass as bass
import concourse.mybir as mybir
import concourse.tile as tile
from concourse.bass_utils import run_bass_kernel_spmd

B, N, D = 2, 2048, 1024
H, HD = 16, 64
NCORES = 8
HPC = H // NCORES      # heads per core = 2
NT = N // 128          # 16 blocks of 128 along seq
NCH = N // 512         # 4 column chunks of 512 along seq
QS = N // 4            # query rows owned per core = 512
QSO = N // NCORES      # query rows owned per core per batch = 256
KT = D // 128          # 8 contraction tiles for d
BF16 = mybir.dt.bfloat16
F32 = mybir.dt.float32
SHIFT = 6.0            # static upper bound of the adjusted logits


def _split_multi_waits(nc):
    """This image's walrus rejects >1 sync-wait per instruction; move extra
    waits onto single-wait NoOps spliced just before the instruction in the
    same engine stream (the engine blocks on the NoOps first)."""
    n_split = 0
    for f in nc.m.functions:
        for bb in f.blocks:
            insts = list(bb.instructions)
            new = []
            for inst in insts:
                si = getattr(inst, "sync_info", None)
                waits = list(si.on_wait) if si is not None and si.on_wait else []
                if len(waits) > 1:
                    for idx, w in enumerate(waits[1:]):
                        nop = mybir.InstNoOp(
                            name=f"{inst.name}-xw{idx}", ins=[], outs=[])
                        nop.engine = inst.engine
                        nop.sync_info = bass_rust.SyncInfo(
                            on_wait=[w], on_update=[])
                        new.append(nop)
                    si.on_wait = waits[:1]
                    n_split += 1
                new.append(inst)
            if len(new) != len(insts):
                bb.instructions = new
    return n_split


def _get_slopes(n):
    def pow2(n):
        start = 2 ** (-(2 ** (-(math.log2(n) - 3))))
        return [start * start**i for i in range(n)]

    if math.log2(n).is_integer():
        return pow2(n)
    c = 2 ** math.floor(math.log2(n))
    return pow2(c) + _get_slopes(2 * c)[0::2][: n - c]


def _chunks_for_kb(kb):
    """512-aligned chunk list [(c0, cw), ...] covering [kb*128, N)."""
    q0 = kb * 128
    out = []
    c0 = q0
    while c0 < N:
        end = min((c0 // 512 + 1) * 512, N)
        out.append((c0, end - c0))
        c0 = end
    return out


def build_nc():
    nc = bass.Bass()

    xT = nc.declare_dram_parameter("xT", [B, 128, NCH * KT * 512], BF16,
                                   isOutput=False)
    wq = nc.declare_dram_parameter("wq", [128, KT * 128], BF16, isOutput=False)
    wk = nc.declare_dram_parameter("wk", [128, KT * 128], BF16, isOutput=False)
    wv = nc.declare_dram_parameter("wv", [128, KT * 128], BF16, isOutput=False)
    wo = nc.declare_dram_parameter("wo", [128, KT * D], BF16, isOutput=False)
    qrow = nc.declare_dram_parameter("qrow", [HPC, N], BF16, isOutput=False)
    kbias = nc.declare_dram_parameter("kbias", [128, HPC * NT], F32,
                                      isOutput=False)
    maskp = nc.declare_dram_parameter("maskp", [128, 128], BF16,
                                      isOutput=False)
    y = nc.declare_dram_parameter("y", [QS, D], F32, isOutput=True)

    # batch 0: one AllToAll; batch 1: one per head (the last is covered by
    # batch 0's output projection)
    a2a_in0 = nc.dram_tensor("a2a_in0", [NCORES, 128, QSO], BF16)
    a2a_out0 = nc.dram_tensor("a2a_out0", [NCORES, 128, QSO], BF16)
    a2a_in1 = [nc.dram_tensor(f"a2a_in1{h}", [NCORES, 64, QSO], BF16)
               for h in range(HPC)]
    a2a_out1 = [nc.dram_tensor(f"a2a_out1{h}", [NCORES, 64, QSO], BF16)
                for h in range(HPC)]
    dum_in = nc.dram_tensor("dum_in", [NCORES, 1, 8], BF16)
    dum_out = nc.dram_tensor("dum_out", [NCORES, 1, 8], BF16)
    # DRAM scratch for the denominator-row reshape (SBUF APs cannot merge
    # partition and free dims; DRAM APs are unconstrained)
    sden = [[nc.dram_tensor(f"sden{b}{h}", [1, N], BF16) for h in range(HPC)]
            for b in range(B)]
    srec = [[nc.dram_tensor(f"srec{b}{h}", [1, N], BF16) for h in range(HPC)]
            for b in range(B)]
    groups = [list(range(NCORES))]

    from contextlib import ExitStack

    with tile.TileContext(nc) as tc, ExitStack() as est:
        cpool = est.enter_context(tc.tile_pool(name="const", bufs=1))
        xpool = est.enter_context(tc.tile_pool(name="x", bufs=1))
        qkpool = est.enter_context(tc.tile_pool(name="qk", bufs=1))
        vpool = est.enter_context(tc.tile_pool(name="v", bufs=1))
        ppool = est.enter_context(tc.tile_pool(name="p", bufs=8))
        rpool = est.enter_context(tc.tile_pool(name="rc", bufs=2))
        ctpool = est.enter_context(tc.tile_pool(name="ct", bufs=1))
        cfpool = est.enter_context(tc.tile_pool(name="cf", bufs=1))
        opool = est.enter_context(tc.tile_pool(name="ob", bufs=2))
        atps = est.enter_context(tc.tile_pool(name="at", bufs=2, space="PSUM"))
        pjps = est.enter_context(tc.tile_pool(name="pj", bufs=2, space="PSUM"))
        ctxps = est.enter_context(tc.tile_pool(name="cx", bufs=1, space="PSUM"))

        # warm-up collective first: kicks the one-time CC-channel init off
        # the critical path (the GpSimd queue carries only collectives)
        ones64 = cpool.tile([1, 64], BF16, tag="ones64", name="ones64")
        nc.vector.memset(ones64[:], 1.0)
        nc.sync.dma_start(out=dum_in[:].rearrange("j p q -> p (j q)"),
                          in_=ones64[:])
        nc.gpsimd.collective_compute(
            "AllToAll", mybir.AluOpType.bypass, replica_groups=groups,
            ins=[dum_in[:].opt()], outs=[dum_out[:].opt()],
        )

        mask = cpool.tile([128, 128], BF16, tag="mask", name="mask")
        nc.sync.dma_start(out=mask[:], in_=maskp[:])
        wq_sb = cpool.tile([128, KT * 128], BF16, tag="wq", name="wq_sb")
        nc.sync.dma_start(out=wq_sb[:], in_=wq[:])
        wk_sb = cpool.tile([128, KT * 128], BF16, tag="wk", name="wk_sb")
        nc.sync.dma_start(out=wk_sb[:], in_=wk[:])
        wv_sb = cpool.tile([128, KT * 128], BF16, tag="wv", name="wv_sb")
        nc.sync.dma_start(out=wv_sb[:], in_=wv[:])
        kb_sb = cpool.tile([128, HPC * NT], F32, tag="kb", name="kb_sb")
        nc.sync.dma_start(out=kb_sb[:], in_=kbias[:])

        # x tiles, chunk-contiguous: cols [ch*KT*512 + kt*512 + j]
        x_t = []
        for b in range(B):
            xt = xpool.tile([128, NCH * KT * 512], BF16, tag=f"xt{b}",
                            name=f"x_t{b}")
            x_t.append(xt)
            for ch in range(NCH):
                cs = slice(ch * KT * 512, (ch + 1) * KT * 512)
                nc.sync.dma_start(out=xt[:, cs], in_=xT[b][:, cs])

        # wo is needed last; keep its 2 MB off the early DMA window
        wo_sb = cpool.tile([128, KT * D], BF16, tag="wo", name="wo_sb")
        nc.sync.dma_start(out=wo_sb[:], in_=wo[:])

        def xcol(b, ch, kt):
            base = ch * KT * 512 + kt * 512
            return x_t[b][:, base:base + 512]

        ct = [ctpool.tile([128, N], BF16, tag=f"ct{b}", name=f"ct{b}")
              for b in range(B)]
        cf = [cfpool.tile([128, NCORES * QSO], BF16, tag=f"cf{b}",
                          name=f"cf{b}")
              for b in range(B)]
        tiles = {}  # b -> (qe, ke, v_t), filled by proj_stream

        def proj_stream(b, act_copies):
            """Projections for batch b.  Yields after ~2 matmuls so it can
            be interleaved with an attention stream."""
            qe = [qkpool.tile([65, N], BF16, tag=f"qe{b}{h}", name=f"qe{b}{h}")
                  for h in range(HPC)]
            ke = [qkpool.tile([65, N], BF16, tag=f"ke{b}{h}", name=f"ke{b}{h}")
                  for h in range(HPC)]
            v_t = [vpool.tile([128, HPC * 65], BF16, tag=f"v{b}_{nb}",
                              name=f"v{b}_{nb}")
                   for nb in range(NT)]
            tiles[b] = (qe, ke, v_t)
            for h in range(HPC):
                nc.sync.dma_start(out=qe[h][64:65, :], in_=qrow[h:h + 1, :])
                nc.vector.memset(ke[h][64:65, :], 1.0)
            cop = nc.scalar.copy if act_copies else nc.vector.tensor_copy
            for w_sb, dst in ((wq_sb, qe), (wk_sb, ke)):
                for ch in range(NCH):
                    ps = pjps.tile([128, 512], F32, tag="pj", name="ps")
                    for kt in range(0, KT, 2):
                        nc.tensor.matmul(
                            ps[:], lhsT=w_sb[:, kt * 128:(kt + 1) * 128],
                            rhs=xcol(b, ch, kt),
                            start=(kt == 0), stop=False,
                        )
                        nc.tensor.matmul(
                            ps[:], lhsT=w_sb[:, (kt + 1) * 128:(kt + 2) * 128],
                            rhs=xcol(b, ch, kt + 1),
                            start=False, stop=(kt == KT - 2),
                        )
                        yield
                    cs = slice(ch * 512, (ch + 1) * 512)
                    cop(dst[0][0:64, cs], ps[0:64, :])
                    cop(dst[1][0:64, cs], ps[64:128, :])
            for nb in range(NT):
                ps = pjps.tile([128, 512], F32, tag="pj", name="vps")
                for kt in range(0, KT, 2):
                    for k2 in (kt, kt + 1):
                        nc.tensor.matmul(
                            ps[:, 0:128],
                            lhsT=xcol(b, nb // 4, k2)[:, (nb % 4) * 128:
                                                      (nb % 4) * 128 + 128],
                            rhs=wv_sb[:, k2 * 128:(k2 + 1) * 128],
                            start=(k2 == 0), stop=(k2 == KT - 1),
                        )
                    yield
                vr = v_t[nb][:].rearrange("p (g c) -> p g c", g=HPC)
                sr = ps[:, 0:128].rearrange("p (g c) -> p g c", g=HPC)
                nc.vector.tensor_copy(vr[:, :, 0:64], sr[:])
                nc.vector.memset(vr[:, :, 64:65], 1.0)

        def attn_stream(b):
            """Attention for batch b.  P@V is issued one k-block behind
            scores (software pipeline); yields after each PE instruction."""
            qe, ke, v_t = tiles[b]
            for h in range(HPC):
                ctx = ctxps.tile([128, N], F32, tag="ctx", name="ctx")
                vsl = slice(h * 65, (h + 1) * 65)

                def scores_exp(kb):
                    q0 = kb * 128
                    col = h * NT + kb
                    pts = []
                    for ci, (c0, cw) in enumerate(_chunks_for_kb(kb)):
                        ps = atps.tile([128, 512], F32, tag="at", name="sps")
                        nc.tensor.matmul(
                            ps[:, 0:cw],
                            lhsT=ke[h][:, q0:q0 + 128],
                            rhs=qe[h][:, c0:c0 + cw],
                            start=True, stop=True,
                        )
                        p_t = ppool.tile([128, 512], BF16, tag="p", name="p_t")
                        pts.append(p_t)
                        nc.scalar.activation(
                            p_t[:, 0:cw], ps[:, 0:cw],
                            mybir.ActivationFunctionType.Exp,
                            bias=kb_sb[:, col:col + 1], scale=1.0,
                        )
                        if ci == 0:  # causal mask on the diagonal block
                            nc.vector.tensor_tensor(
                                p_t[:, 0:128], p_t[:, 0:128], mask[:],
                                op=mybir.AluOpType.mult,
                            )
                        yield pts

                def pv(kb, pts):
                    q0 = kb * 128
                    chunks = _chunks_for_kb(kb)
                    if kb == 0:
                        for ci, (c0, cw) in enumerate(chunks):
                            nc.tensor.matmul(
                                ctx[0:65, c0:c0 + cw],
                                lhsT=v_t[kb][:, vsl], rhs=pts[ci][:, 0:cw],
                                start=True, stop=False,
                                skip_group_check=True,
                            )
                            yield
                        return
                    c00, cw0 = chunks[0]
                    if cw0 > 128:  # first chunk minus the diagonal block
                        nc.tensor.matmul(
                            ctx[0:65, c00 + 128:c00 + cw0],
                            lhsT=v_t[kb][:, vsl], rhs=pts[0][:, 128:cw0],
                            start=False, stop=False,
                            skip_group_check=True,
                        )
                        yield
                    for ci, (c0, cw) in enumerate(chunks[1:], 1):
                        nc.tensor.matmul(
                            ctx[0:65, c0:c0 + cw],
                            lhsT=v_t[kb][:, vsl], rhs=pts[ci][:, 0:cw],
                            start=False, stop=False,
                            skip_group_check=True,
                        )
                        yield
                    nc.tensor.matmul(  # diagonal block: final write
                        ctx[0:65, q0:q0 + 128],
                        lhsT=v_t[kb][:, vsl], rhs=pts[0][:, 0:128],
                        start=False, stop=True,
                        skip_group_check=True,
                    )
                    yield

                prev = None
                for kb in range(NT):
                    pts = None
                    for pts in scores_exp(kb):
                        yield
                    if prev is not None:
                        for _ in pv(*prev):
                            yield
                    prev = (kb, pts)
                for _ in pv(*prev):
                    yield

                # normalize: ct[h rows, q] = ctx[0:64, q] * (1/ctx[64, q]).
                # The DVE's iterative-divide reciprocal costs ~8 cyc per
                # FREE-dim element, so fold the 2048 denominators to
                # [128, 16] with a tiny DMA first (recip there is ~130 ns,
                # not 13 us), DMA back to a row, then broadcast via a
                # rank-1 matmul (projection-pool PSUM so the attention
                # score pipeline is not coupled to extraction), stage to
                # SBUF and multiply.
                den = rpool.tile([1, N], BF16, tag="den", name="den")
                nc.scalar.copy(den[:], ctx[64:65, :])
                nc.sync.dma_start(out=sden[b][h][:], in_=den[:])
                dr = rpool.tile([128, 16], BF16, tag="dr", name="dr")
                nc.sync.dma_start(
                    out=dr[:],
                    in_=sden[b][h][:].rearrange("o (p j) -> (o p) j", p=128),
                )
                rr32 = rpool.tile([128, 16], F32, tag="rr32", name="rr32")
                nc.vector.reciprocal(rr32[:], dr[:])
                rr16 = rpool.tile([128, 16], BF16, tag="rr16", name="rr16")
                nc.vector.tensor_copy(rr16[:], rr32[:])
                nc.sync.dma_start(
                    out=srec[b][h][:].rearrange("o (p j) -> (o p) j", p=128),
                    in_=rr16[:],
                )
                rrow = rpool.tile([1, N], BF16, tag="rrow", name="rrow")
                nc.sync.dma_start(out=rrow[:], in_=srec[b][h][:])
                for c0 in range(0, N, 512):
                    bc = pjps.tile([128, 512], F32, tag="pj", name="bc")
                    nc.tensor.matmul(
                        bc[0:64, :], lhsT=ones64[:],
                        rhs=rrow[:, c0:c0 + 512], start=True, stop=True,
                    )
                    yield
                    bcs = rpool.tile([64, 512], BF16, tag="bcs", name="bcs")
                    nc.vector.tensor_copy(bcs[:], bc[0:64, :])
                    nc.vector.tensor_tensor(
                        ct[b][h * 64:(h + 1) * 64, c0:c0 + 512],
                        ctx[0:64, c0:c0 + 512], bcs[:],
                        op=mybir.AluOpType.mult,
                    )
                # stage + collective(s)
                if b == 0:
                    if h == HPC - 1:
                        nc.sync.dma_start(
                            out=a2a_in0[:].rearrange("j p q -> p j q"),
                            in_=ct[0][:].rearrange("p (j q) -> p j q",
                                                   j=NCORES),
                        )
                        nc.gpsimd.collective_compute(
                            "AllToAll", mybir.AluOpType.bypass,
                            replica_groups=groups,
                            ins=[a2a_in0[:].opt()], outs=[a2a_out0[:].opt()],
                        )
                        nc.sync.dma_start(
                            out=cf[0][:].rearrange("p (j q) -> p j q",
                                                   j=NCORES),
                            in_=a2a_out0[:].rearrange("j p q -> p j q"),
                        )
                else:
                    nc.sync.dma_start(
                        out=a2a_in1[h][:].rearrange("j p q -> p j q"),
                        in_=ct[1][h * 64:(h + 1) * 64, :].rearrange(
                            "p (j q) -> p j q", j=NCORES),
                    )
                    nc.gpsimd.collective_compute(
                        "AllToAll", mybir.AluOpType.bypass,
                        replica_groups=groups,
                        ins=[a2a_in1[h][:].opt()],
                        outs=[a2a_out1[h][:].opt()],
                    )
                    nc.sync.dma_start(
                        out=cf[1][h * 64:(h + 1) * 64, :].rearrange(
                            "p (j q) -> p j q", j=NCORES),
                        in_=a2a_out1[h][:].rearrange("j p q -> p j q"),
                    )

        def out_proj(b):
            for q4 in range(QSO // 128):
                ob = opool.tile([128, D], F32, tag="ob", name="ob")
                for nch in range(D // 512):
                    ps = pjps.tile([128, 512], F32, tag="pj", name="wps")
                    for kt in range(KT):
                        nc.tensor.matmul(
                            ps[:],
                            lhsT=cf[b][:, kt * QSO + q4 * 128:
                                       kt * QSO + (q4 + 1) * 128],
                            rhs=wo_sb[:, kt * D + nch * 512:
                                      kt * D + (nch + 1) * 512],
                            start=(kt == 0), stop=(kt == KT - 1),
                        )
                    nc.vector.tensor_copy(
                        ob[:, nch * 512:(nch + 1) * 512], ps[:])
                r0 = b * QSO + q4 * 128
                nc.sync.dma_start(out=y[r0:r0 + 128, :], in_=ob[:])

        def drain(g):
            for _ in g:
                pass

        def interleave(a, p, ratio=3):
            """Pull `ratio` steps from stream a per step of stream p (the
            attention stream needs ~3 PE instructions issued per exp to keep
            the ScalarE saturated); drain whichever stream survives."""
            a_done = p_done = False
            while not (a_done and p_done):
                for _ in range(ratio):
                    if not a_done:
                        try:
                            next(a)
                        except StopIteration:
                            a_done = True
                if not p_done:
                    try:
                        next(p)
                    except StopIteration:
                        p_done = True

        drain(proj_stream(0, act_copies=True))
        interleave(attn_stream(0), proj_stream(1, act_copies=False))
        drain(attn_stream(1))
        out_proj(0)   # covers batch 1's last collective
        out_proj(1)

    _split_multi_waits(nc)
    return nc


_NC_CACHE = None


def _prep_inputs(x, Wq, Wk, Wv, Wo, bo):
    """Host-side sharding/layout prep. Returns in_maps for the 8 cores."""
    bf = ml_dtypes.bfloat16
    x = np.asarray(x, np.float32)
    slopes = np.array(_get_slopes(H), np.float64)

    # x transposed, chunk-contiguous: xTr[b, p, ch*KT*512 + kt*512 + j]
    #   = x[b, ch*512 + j, kt*128 + p]
    xTr = np.ascontiguousarray(
        x.transpose(0, 2, 1)                     # [B, D, N]
        .reshape(B, KT, 128, NCH, 512)
        .transpose(0, 2, 3, 1, 4)                # [B, 128, NCH, KT, 512]
        .reshape(B, 128, NCH * KT * 512)
    ).astype(bf)

    def wtile(w):  # [D, m] -> [128, KT*m]
        m = w.shape[1]
        return np.ascontiguousarray(
            w.reshape(KT, 128, m).transpose(1, 0, 2).reshape(128, KT * m)
        ).astype(bf)

    # causal keep-mask in S_T layout: 1 where k(partition) <= q(free)
    pp = np.arange(128)
    maskv = (pp[:, None] <= pp[None, :]).astype(bf)

    wo_r = wtile(np.asarray(Wo, np.float32))
    in_maps = []
    for c in range(NCORES):
        hs = slice(c * HPC * HD, (c + 1) * HPC * HD)
        sl = slopes[c * HPC:(c + 1) * HPC] / 8.0
        q_idx = np.arange(N, dtype=np.float64)
        qr = (-sl[:, None] * q_idx[None, :] - SHIFT).astype(bf)
        p = np.arange(128, dtype=np.float64)
        kb = np.zeros((128, HPC * NT), np.float32)
        for h in range(HPC):
            for t in range(NT):
                kb[:, h * NT + t] = (sl[h] * (t * 128 + p)).astype(np.float32)
        in_maps.append({
            "xT": xTr,
            "wq": wtile(np.asarray(Wq, np.float32)[:, hs] / 8.0),
            "wk": wtile(np.asarray(Wk, np.float32)[:, hs]),
            "wv": wtile(np.asarray(Wv, np.float32)[:, hs]),
            "wo": wo_r,
            "qrow": qr,
            "kbias": kb,
            "maskp": maskv,
        })
    return in_maps


def _try_device_reset():
    """Best-effort NeuronCore reset via the axon client (clears collective
    state a previously killed run may have left behind)."""
    try:
        import ctypes
        import time as _time

        import jax

        jax.devices()
        lib = ctypes.CDLL("/opt/axon/libaxon_pjrt.so")
        lib.axon_reset.restype = ctypes.c_int64
        lib.axon_reset()
        _time.sleep(5)
    except Exception:
        pass


def kernel(x, Wq, Wk, Wv, Wo, bo):
    global _NC_CACHE
    if _NC_CACHE is None:
        _NC_CACHE = build_nc()
    nc = _NC_CACHE
    in_maps = _prep_inputs(x, Wq, Wk, Wv, Wo, bo)
    try:
        res = run_bass_kernel_spmd(nc, in_maps, list(range(NCORES)))
    except Exception:
        _try_device_reset()
        res = run_bass_kernel_spmd(nc, in_maps, list(range(NCORES)))
    out = np.empty((B, N, D), np.float32)
    for c in range(NCORES):
        for b in range(B):
            out[b, c * QSO:(c + 1) * QSO, :] = \
                res.results[c]["y"][b * QSO:(b + 1) * QSO]
    out += np.asarray(bo, np.float32)[None, None, :]
    return out
